# revision 1
# baseline (speedup 1.0000x reference)
"""Trainium2 Bass kernel for a dense GAT layer (B=4, N=2048, FIN=128, K=4 heads, D=32).

Math (per batch b):
    Wh = (H @ W).reshape(N, K, D)
    s[i,k] = <Wh[i,k,:], a_src[k,:]>;  t[j,k] = <Wh[j,k,:], a_dst[k,:]>
    e[i,j,k] = leaky_relu(s[i,k] + t[j,k], 0.2), masked to -inf where A[i,j] == 0
    alpha = softmax_j(e);  out[i] = sum_j alpha[i,j,k] * Wh[j,k,:]

Kernel reformulation (exact in exact arithmetic):
    exp(lrelu(x)) = max(exp(x), exp(0.2 x)); with x = s_i + t_j both branches are
    rank-1, and the i-side factor exp(0.2 s_i) cancels in the softmax. So with
    G_i = exp(0.8 s_i), H_j = exp(0.8 t_j), F2_j = exp(0.2 t_j), m = (A > 0):
        w[j,i]   = m[i,j] * max(G_i * H_j, 1)
        out[i,:] = (sum_j w[j,i] * F2_j * Wh[j,:]) / (sum_j w[j,i] * F2_j)
    Scores live in transposed [j (partitions), i (free)] layout so the
    j-contraction runs on the tensor engine with PSUM accumulation; appending F2
    as an extra column of the stationary operand yields the denominators free.

Sharding: 8 cores = 4 batches x 2 row-halves. The host rotates each core's H
rows / A columns so its own query rows are always local rows 0..1023 (keeps the
SPMD program identical across cores), and ships H and A pre-transposed so the
device needs no fp32 transposes for them.
"""

import numpy as np
from contextlib import ExitStack

import concourse.bacc as bacc
import concourse.mybir as mybir
import concourse.tile as tile
from concourse.bass_utils import run_bass_kernel_spmd

B, N, FIN = 4, 2048, 128
KH, DH = 4, 32
P = 128
NI = 1024  # query rows per core
JT = N // P  # 16 j-chunks
NIB = 2  # i-blocks per core
IBS = NI // NIB  # 512
ICN = IBS // P  # 4 i-chunks of 128 per block
JP = 2  # j-chunks paired per mask-multiply op

f32 = mybir.dt.float32
bf16 = mybir.dt.bfloat16

_CACHE = {}


def _build_program():
    nc = bacc.Bacc("TRN2", target_bir_lowering=False, debug=False)

    def din(name, shape, dtype=f32):
        return nc.dram_tensor(name, list(shape), dtype, kind="ExternalInput").ap()

    wu_d = din("wu", (P, 8 + P))       # tiny tensor: first DMA, PE warm-up fodder
    AT_d = din("AslabT", (N, NI))      # A slab transposed: [j, i]
    sel_d = din("sel", (KH, KH * P))   # head-selector for the Gb0 broadcast
    gscr_d = nc.dram_tensor("gscr", [KH, NI], f32).ap()  # Grow bounce for bcast
    CPW = KH * DH + 2 * KH + P + N  # [W | Ssrc | Sdst | ident | HT]
    cpack_d = din("cpack", (P, CPW))
    oaux_d = nc.dram_tensor(
        "oaux", [NIB, KH, DH + 1, IBS], f32, kind="ExternalOutput"
    ).ap()

    Exp = mybir.ActivationFunctionType.Exp
    Sign = mybir.ActivationFunctionType.Sign
    Copy = mybir.ActivationFunctionType.Copy
    MULT = mybir.AluOpType.mult
    MAX = mybir.AluOpType.max

    with tile.TileContext(nc) as tc, ExitStack() as ctx:
        const = ctx.enter_context(tc.tile_pool(name="const", bufs=1))
        big = ctx.enter_context(tc.tile_pool(name="big", bufs=1))
        dbuf = ctx.enter_context(tc.tile_pool(name="dbuf", bufs=2))
        astg = ctx.enter_context(tc.tile_pool(name="astg", bufs=3))
        work = ctx.enter_context(tc.tile_pool(name="work", bufs=2))
        small = ctx.enter_context(tc.tile_pool(name="small", bufs=2))
        ps = ctx.enter_context(tc.tile_pool(name="ps", bufs=3, space="PSUM"))
        pspv = ctx.enter_context(tc.tile_pool(name="pspv", bufs=1, space="PSUM"))

        # ---- constants / inputs ----
        wu = const.tile([P, 8 + P], f32, tag="wu")
        nc.sync.dma_start(wu[:], wu_d[:])
        cpack = const.tile([P, CPW], f32, tag="cpack")
        nc.sync.dma_start(cpack[:], cpack_d[:])
        sel = const.tile([KH, KH * P], f32, tag="sel")
        nc.sync.dma_start(sel[:], sel_d[:])
        sbW = cpack[:, 0:P]
        ssrc = cpack[:, P:P + KH]
        sdst = cpack[:, P + KH:P + 2 * KH]
        ident = cpack[:, P + 2 * KH:2 * P + 2 * KH]
        HT = cpack[:, 2 * P + 2 * KH:]  # [fin, n]
        # PE warm-up: junk transposes on the tiny first tensor so the HAM
        # clock ramps while the big input DMA is still in flight
        for _ in range(24):
            pwu = ps.tile([P, 512], f32, tag="stg")
            nc.tensor.transpose(pwu[0:8, 0:P], wu[:, 0:8], wu[:, 8:8 + P])

        # ---- mask compare helpers ----
        IS_GT = mybir.AluOpType.is_gt

        # ---- stage 0a: WhT and the chain gating the y-scores (srow/Grow/Gb) ----
        WhT = big.tile([P, N], f32, tag="WhT")  # [kd, n] = (H @ W).T
        for q in range(N // 512):
            pw = ps.tile([P, 512], f32, tag="stg")
            nc.tensor.matmul(pw[:], sbW, HT[:, q * 512:(q + 1) * 512],
                             start=True, stop=True)
            nc.scalar.copy(WhT[:, q * 512:(q + 1) * 512], pw[:])

        srow = big.tile([KH, NI], f32, tag="srow")  # s in [k, i] rows (own rows)
        for q in range(NI // 512):
            ps3 = ps.tile([P, 512], f32, tag="stg")
            nc.tensor.matmul(ps3[0:KH, :], ssrc, WhT[:, q * 512:(q + 1) * 512],
                             start=True, stop=True)
            nc.scalar.copy(srow[:, q * 512:(q + 1) * 512], ps3[0:KH, :])

        Grow = big.tile([KH, NI], f32, tag="Grow")  # exp(0.8 s)
        nc.scalar.activation(Grow[:], srow[:], Exp, scale=0.8)
        nc.sync.dma_start(gscr_d[:], Grow[:])  # bounce to DRAM for broadcast reads

        # G broadcast rows per i-block via stride-0 DRAM reads:
        # Gb[:, k, i] = exp(0.8 s)[k, i] replicated across partitions
        def g_block(ib):
            isl = slice(ib * IBS, (ib + 1) * IBS)
            Gb = dbuf.tile([P, KH, IBS], f32, tag="Gb", name=f"Gb{ib}")
            for k in range(KH):
                nc.sync.dma_start(Gb[:, k, :], gscr_d[k, isl].partition_broadcast(P))
            return Gb

        # block 0 via PE matmul (PE is prologue-idle and this skips the DRAM
        # round-trip latency); block 1 via the DMA broadcast above
        Gb0 = dbuf.tile([P, KH, IBS], f32, tag="Gb", name="Gb0")
        for k in range(KH):
            pg = ps.tile([P, 512], f32, tag="stg")
            nc.tensor.matmul(pg[:], sel[:, k * P:(k + 1) * P], Grow[:, 0:IBS],
                             start=True, stop=True)
            nc.scalar.copy(Gb0[:, k, :], pg[:])

        # ---- stage 0b (per j-chunk): t-factors, PV stationary, and the mask ----
        # Hcol/F2col = exp(0.8 t)/exp(0.2 t) straight from the t PSUM tile;
        # whf[jt][:, k, :] = [Wh_k * F2 | F2]; mask compare on DVE (block 0,
        # prologue-idle) and ACT (block 1).
        Hcol = big.tile([P, JT, KH], f32, tag="Hcol")
        F2col = big.tile([P, JT, KH], f32, tag="F2col")
        whf = []
        mT0, mT1 = [], []
        m0 = m1 = None
        for jt in range(JT):
            pt2 = ps.tile([P, 512], f32, tag="stg")
            nc.tensor.matmul(pt2[:, 0:KH], WhT[:, jt * P:(jt + 1) * P], sdst,
                             start=True, stop=True)
            nc.scalar.activation(Hcol[:, jt, :], pt2[:, 0:KH], Exp, scale=0.8)
            nc.scalar.activation(F2col[:, jt, :], pt2[:, 0:KH], Exp, scale=0.2)

            wt = big.tile([P, KH, DH + 1], f32, tag=f"whf{jt}", name=f"whf{jt}")
            pn = ps.tile([P, 512], f32, tag="stg")
            nc.tensor.transpose(pn[:, 0:P], WhT[:, jt * P:(jt + 1) * P], ident)
            for k in range(KH):
                nc.scalar.activation(
                    wt[:, k, 0:DH], pn[:, k * DH:(k + 1) * DH], Copy,
                    scale=F2col[:, jt, k:k + 1],
                )
            nc.scalar.copy(wt[:, :, DH:DH + 1], F2col[:, jt, :, None])
            whf.append(wt)

            r = jt % JP
            if r == 0:
                m0 = dbuf.tile([P, JP, IBS], bf16, tag="mTp", bufs=2 * (JT // JP),
                               name=f"mT0_{jt // JP}")
                m1 = dbuf.tile([P, JP, IBS], bf16, tag="mTp", bufs=2 * (JT // JP),
                               name=f"mT1_{jt // JP}")
                mT0.append(m0)
                mT1.append(m1)
            aT = astg.tile([P, NI], f32, tag="aT")
            nc.sync.dma_start(aT[:], AT_d[jt * P:(jt + 1) * P, :])
            nc.vector.tensor_scalar(m0[:, r, :], aT[:, 0:IBS], 0.0, None, IS_GT)
            nc.scalar.activation(m1[:, r, :], aT[:, IBS:NI], Sign)

        Gb1 = g_block(1)

        # ---- main loop over i-blocks ----
        for ib, (mTs, Gb) in enumerate([(mT0, Gb0), (mT1, Gb1)]):
            # PV accumulators (transposed): per head [33, i] = [WhF2|F2].T @ w
            # one full PSUM bank per head -> single live accumulation group/bank
            pv = [
                pspv.tile([DH + 1, IBS], f32, tag=f"pv{k}", name=f"pv{k}_{ib}")
                for k in range(KH)
            ]

            for jp in range(JT // JP):
                y8 = work.tile([P, JP, KH, IBS], f32, tag="y8")
                for r in range(JP):
                    jt = jp * JP + r
                    for k in range(KH):
                        eng = nc.vector if k < 2 else nc.gpsimd
                        eng.tensor_scalar(
                            y8[:, r, k, :], Gb[:, k, :], Hcol[:, jt, k:k + 1], 1.0,
                            MULT, MAX,
                        )
                for r in range(JP):
                    nc.vector.tensor_mul(
                        y8[:, r], y8[:, r],
                        mTs[jp][:, r, None, :].broadcast_to((P, KH, IBS)),
                    )
                for r in range(JP):
                    jt = jp * JP + r
                    for k in range(KH):
                        nc.tensor.matmul(
                            pv[k][:],
                            whf[jt][:, k, :],
                            y8[:, r, k, :],
                            start=(jt == 0),
                            stop=(jt == JT - 1),
                        )

            # epilogue: ship the raw [33, i] accumulators (numerators + the
            # denominator row); the host performs the divide and un-transpose
            otT = small.tile([DH + 1, KH, IBS], f32, tag="otT")
            for k in range(KH):
                if k < 2:
                    nc.scalar.copy(otT[:, k, :], pv[k][:])
                else:
                    nc.vector.tensor_copy(otT[:, k, :], pv[k][:])
            nc.sync.dma_start(oaux_d[ib].rearrange("k d i -> d k i"), otT[:])

    nc.compile()
    return nc


def _host_prep(H, A, W, a_src, a_dst):
    """Build the 8 per-core input maps (layout prep only)."""
    Ssrc = np.zeros((FIN, KH), np.float32)
    Sdst = np.zeros((FIN, KH), np.float32)
    for k in range(KH):
        Ssrc[k * DH:(k + 1) * DH, k] = a_src[k]
        Sdst[k * DH:(k + 1) * DH, k] = a_dst[k]

    in_maps = []
    for c in range(8):
        b, half = divmod(c, 2)
        i0 = half * NI
        HbT = np.roll(H[b], -i0, axis=0).T.astype(np.float32)
        AslabT = np.ascontiguousarray(
            np.roll(A[b, i0:i0 + NI, :], -i0, axis=1).T
        ).astype(np.float32)
        cpack = np.concatenate(
            [W.astype(np.float32), Ssrc, Sdst, np.eye(P, dtype=np.float32), HbT],
            axis=1,
        )
        sel = np.zeros((KH, KH * P), np.float32)
        for k in range(KH):
            sel[k, k * P:(k + 1) * P] = 1.0
        wu_host = np.ones((P, 8 + P), np.float32)
        wu_host[:, 8:] = np.eye(P, dtype=np.float32)
        in_maps.append({
            "AslabT": AslabT,
            "cpack": np.ascontiguousarray(cpack),
            "wu": wu_host,
            "sel": sel,
        })
    return in_maps


def kernel(H, A, W, a_src, a_dst, _want_results=False, _trace=False):
    H = np.asarray(H); A = np.asarray(A); W = np.asarray(W)
    a_src = np.asarray(a_src); a_dst = np.asarray(a_dst)

    if "nc" not in _CACHE:
        _CACHE["nc"] = _build_program()
    nc = _CACHE["nc"]

    in_maps = _host_prep(H, A, W, a_src, a_dst)
    res = run_bass_kernel_spmd(nc, in_maps, list(range(8)), trace=_trace)

    out = np.empty((B, N, KH * DH), np.float32)
    for c in range(8):
        b, half = divmod(c, 2)
        i0 = half * NI
        aux = res.results[c]["oaux"]  # [NIB, KH, DH+1, IBS]
        slab = aux[:, :, 0:DH, :] / aux[:, :, DH:DH + 1, :]
        # [ib, k, d, i] -> rows (ib*IBS + i), cols (k*DH + d)
        out[b, i0:i0 + NI, :] = (
            slab.transpose(0, 3, 1, 2).reshape(NI, KH * DH)
        )
    if _want_results:
        return out, res
    return out



# revision 5
# speedup vs baseline: 1.5697x; 1.5697x over previous
"""Trainium2 Bass kernel for a dense GAT layer (B=4, N=2048, FIN=128, K=4 heads, D=32).

Math (per batch b):
    Wh = (H @ W).reshape(N, K, D)
    s[i,k] = <Wh[i,k,:], a_src[k,:]>;  t[j,k] = <Wh[j,k,:], a_dst[k,:]>
    e[i,j,k] = leaky_relu(s[i,k] + t[j,k], 0.2), masked to -inf where A[i,j] == 0
    alpha = softmax_j(e);  out[i] = sum_j alpha[i,j,k] * Wh[j,k,:]

Kernel reformulation (exact in exact arithmetic):
    exp(lrelu(x)) = max(exp(x), exp(0.2 x)); with x = s_i + t_j both branches are
    rank-1, and the i-side factor exp(0.2 s_i) cancels in the softmax. So with
    G_i = exp(0.8 s_i), H_j = exp(0.8 t_j), F2_j = exp(0.2 t_j), m = (A > 0):
        w[j,i]   = m[i,j] * max(G_i * H_j, 1)
        out[i,:] = (sum_j w[j,i] * F2_j * Wh[j,:]) / (sum_j w[j,i] * F2_j)
    Scores live in transposed [j (partitions), i (free)] layout so the
    j-contraction runs on the tensor engine with PSUM accumulation; appending F2
    as an extra column of the stationary operand yields the denominators free.

Everything on the score path is bf16 (tolerance is 2e-2): the PE runs matmuls at
1 cycle/row instead of fp32's 4, the DVE hits its 4x (tensor_scalar) and 2x
(tensor_tensor) perf modes, and the mask ships from the host as a ready-made
bf16 0/1 tile so no compare pass runs on device. The mask multiply is split
DVE/Pool to balance the two engines.

Sharding: 8 cores = 4 batches x 2 row-halves. The host rotates each core's H
rows / A columns so its own query rows are always local rows 0..1023 (keeps the
SPMD program identical across cores), and ships H and the mask pre-transposed so
the device needs no fp32 transposes for them.
"""

import numpy as np
from contextlib import ExitStack

import concourse.bacc as bacc
import concourse.mybir as mybir
import concourse.tile as tile
from concourse.bass_utils import run_bass_kernel_spmd

B, N, FIN = 4, 2048, 128
KH, DH = 4, 32
P = 128
NI = 1024  # query rows per core
JT = N // P  # 16 j-chunks
NIB = 2  # i-blocks per core
IBS = NI // NIB  # 512
JP = 2  # j-chunks paired per mask-multiply op

f32 = mybir.dt.float32
bf16 = mybir.dt.bfloat16

_CACHE = {}


def _build_program():
    nc = bacc.Bacc("TRN2", target_bir_lowering=False, debug=False)

    def din(name, shape, dtype=f32):
        return nc.dram_tensor(name, list(shape), dtype, kind="ExternalInput").ap()

    wu_d = din("wu", (P, 8 + P), bf16)   # tiny tensor: first DMA, PE warm-up fodder
    mT_d = din("maskT", (N, NI), bf16)   # 0/1 mask slab transposed: [j, i]
    sel_d = din("sel", (KH, KH * P), bf16)  # head-selector for the Gb0 broadcast
    gscr_d = nc.dram_tensor("gscr", [KH, NI], bf16).ap()  # Grow bounce for bcast
    CPW = KH * DH + 2 * KH + P + N  # [W | Ssrc | Sdst | ident | HT]
    cpack_d = din("cpack", (P, CPW), bf16)
    oaux_d = nc.dram_tensor(
        "oaux", [NIB, KH, DH + 1, IBS], f32, kind="ExternalOutput"
    ).ap()

    Exp = mybir.ActivationFunctionType.Exp
    Copy = mybir.ActivationFunctionType.Copy
    MULT = mybir.AluOpType.mult
    MAX = mybir.AluOpType.max

    with tile.TileContext(nc) as tc, ExitStack() as ctx:
        const = ctx.enter_context(tc.tile_pool(name="const", bufs=1))
        big = ctx.enter_context(tc.tile_pool(name="big", bufs=1))
        dbuf = ctx.enter_context(tc.tile_pool(name="dbuf", bufs=2))
        work = ctx.enter_context(tc.tile_pool(name="work", bufs=2))
        small = ctx.enter_context(tc.tile_pool(name="small", bufs=2))
        ps = ctx.enter_context(tc.tile_pool(name="ps", bufs=3, space="PSUM"))
        pspv = ctx.enter_context(tc.tile_pool(name="pspv", bufs=1, space="PSUM"))

        # ---- constants / inputs ----
        wu = const.tile([P, 8 + P], bf16, tag="wu")
        nc.sync.dma_start(wu[:], wu_d[:])
        cpack = const.tile([P, CPW], bf16, tag="cpack")
        nc.sync.dma_start(cpack[:], cpack_d[:])
        sel = const.tile([KH, KH * P], bf16, tag="sel")
        nc.sync.dma_start(sel[:], sel_d[:])
        sbW = cpack[:, 0:P]
        ssrc = cpack[:, P:P + KH]
        sdst = cpack[:, P + KH:P + 2 * KH]
        ident = cpack[:, P + 2 * KH:2 * P + 2 * KH]
        HT = cpack[:, 2 * P + 2 * KH:]  # [fin, n]
        # PE warm-up: junk transposes on the tiny first tensor so the HAM
        # clock ramps while the big input DMA is still in flight
        for _ in range(24):
            pwu = ps.tile([P, 1024], bf16, tag="stgb", bufs=2)
            nc.tensor.transpose(pwu[0:8, 0:P], wu[:, 0:8], wu[:, 8:8 + P])

        # ---- stage 0a: WhT and the chain gating the y-scores (srow/Grow/Gb) ----
        WhT = big.tile([P, N], bf16, tag="WhT")  # [kd, n] = (H @ W).T
        for q in range(N // 512):
            pw = ps.tile([P, 512], f32, tag="stg", bufs=2)
            nc.tensor.matmul(pw[:], sbW, HT[:, q * 512:(q + 1) * 512],
                             start=True, stop=True)
            nc.scalar.copy(WhT[:, q * 512:(q + 1) * 512], pw[:])

        # s in [k, i] rows (own rows): exp straight out of PSUM into bf16 Grow
        Grow = big.tile([KH, NI], bf16, tag="Grow")  # exp(0.8 s)
        for q in range(NI // 512):
            ps3 = ps.tile([P, 512], f32, tag="stg", bufs=2)
            nc.tensor.matmul(ps3[0:KH, :], ssrc, WhT[:, q * 512:(q + 1) * 512],
                             start=True, stop=True)
            nc.scalar.activation(Grow[:, q * 512:(q + 1) * 512], ps3[0:KH, :],
                                 Exp, scale=0.8)
        nc.sync.dma_start(gscr_d[:], Grow[:])  # bounce to DRAM for broadcast reads

        # G broadcast rows per i-block via stride-0 DRAM reads:
        # Gb[:, k, i] = exp(0.8 s)[k, i] replicated across partitions
        def g_block(ib):
            isl = slice(ib * IBS, (ib + 1) * IBS)
            Gb = dbuf.tile([P, KH, IBS], bf16, tag="Gb", name=f"Gb{ib}")
            for k in range(KH):
                nc.sync.dma_start(Gb[:, k, :], gscr_d[k, isl].partition_broadcast(P))
            return Gb

        # block 0 via PE matmul (PE is prologue-idle and this skips the DRAM
        # round-trip latency); block 1 via the DMA broadcast above
        Gb0 = dbuf.tile([P, KH, IBS], bf16, tag="Gb", name="Gb0")
        for k in range(KH):
            pg = ps.tile([P, 512], f32, tag="stg", bufs=2)
            nc.tensor.matmul(pg[:], sel[:, k * P:(k + 1) * P], Grow[:, 0:IBS],
                             start=True, stop=True)
            nc.scalar.copy(Gb0[:, k, :], pg[:])

        # ---- stage 0b (per j-chunk): t-factors, PV stationary, and the mask ----
        # Hcol/F2col = exp(0.8 t)/exp(0.2 t) straight from the t PSUM tile
        # (fp32: they are per-partition scalar operands, exempt from 2x rules);
        # whf[jt][:, k, :] = [Wh_k * F2 | F2]; mask tiles arrive ready from DRAM.
        Hcol = big.tile([P, JT, KH], f32, tag="Hcol")
        F2col = big.tile([P, JT, KH], f32, tag="F2col")
        whf = []
        mTs = []
        for jt in range(JT):
            pt2 = ps.tile([P, 512], f32, tag="stg", bufs=2)
            nc.tensor.matmul(pt2[:, 0:KH], WhT[:, jt * P:(jt + 1) * P], sdst,
                             start=True, stop=True)
            nc.scalar.activation(Hcol[:, jt, :], pt2[:, 0:KH], Exp, scale=0.8)
            nc.scalar.activation(F2col[:, jt, :], pt2[:, 0:KH], Exp, scale=0.2)

            wt = big.tile([P, KH, DH + 1], bf16, tag=f"whf{jt}", name=f"whf{jt}")
            pn = ps.tile([P, 1024], bf16, tag="stgb", bufs=2)
            nc.tensor.transpose(pn[:, 0:P], WhT[:, jt * P:(jt + 1) * P], ident)
            for k in range(KH):
                nc.scalar.activation(
                    wt[:, k, 0:DH], pn[:, k * DH:(k + 1) * DH], Copy,
                    scale=F2col[:, jt, k:k + 1],
                )
            nc.scalar.copy(wt[:, :, DH:DH + 1], F2col[:, jt, :, None])
            whf.append(wt)

            mt = dbuf.tile([P, NI], bf16, tag="mTp", bufs=JT, name=f"mT{jt}")
            nc.sync.dma_start(mt[:], mT_d[jt * P:(jt + 1) * P, :])
            mTs.append(mt)

        Gb1 = g_block(1)

        # ---- main loop over i-blocks ----
        tt_idx = 0
        for ib, Gb in enumerate([Gb0, Gb1]):
            # PV accumulators (transposed): per head [33, i] = [WhF2|F2].T @ w
            # one full PSUM bank per head -> single live accumulation group/bank
            pv = [
                pspv.tile([DH + 1, IBS], f32, tag=f"pv{k}", name=f"pv{k}_{ib}")
                for k in range(KH)
            ]

            for jp in range(JT // JP):
                y8 = work.tile([P, JP, KH, IBS], bf16, tag="y8")
                for r in range(JP):
                    jt = jp * JP + r
                    for k in range(KH):
                        nc.vector.tensor_scalar(
                            y8[:, r, k, :], Gb[:, k, :], Hcol[:, jt, k:k + 1], 1.0,
                            MULT, MAX,
                        )
                for r in range(JP):
                    jt = jp * JP + r
                    # mask multiply: 0/1 bf16 tile broadcast over heads.
                    # ~3/8 of these go to Pool to offload the DVE.
                    eng = nc.gpsimd if (tt_idx % 8) in (1, 4, 6) else nc.vector
                    tt_idx += 1
                    eng.tensor_mul(
                        y8[:, r], y8[:, r],
                        mTs[jt][:, None, ib * IBS:(ib + 1) * IBS]
                        .broadcast_to((P, KH, IBS)),
                    )
                for r in range(JP):
                    jt = jp * JP + r
                    for k in range(KH):
                        nc.tensor.matmul(
                            pv[k][:],
                            whf[jt][:, k, :],
                            y8[:, r, k, :],
                            start=(jt == 0),
                            stop=(jt == JT - 1),
                        )

            # epilogue: ship the raw [33, i] accumulators (numerators + the
            # denominator row); the host performs the divide and un-transpose
            otT = small.tile([DH + 1, KH, IBS], f32, tag="otT")
            for k in range(KH):
                if k < 2:
                    nc.scalar.copy(otT[:, k, :], pv[k][:])
                else:
                    nc.vector.tensor_copy(otT[:, k, :], pv[k][:])
            nc.sync.dma_start(oaux_d[ib].rearrange("k d i -> d k i"), otT[:])

    nc.compile()
    return nc


def _host_prep(H, A, W, a_src, a_dst):
    """Build the 8 per-core input maps (layout/dtype prep only)."""
    import ml_dtypes
    bf = ml_dtypes.bfloat16

    Ssrc = np.zeros((FIN, KH), np.float32)
    Sdst = np.zeros((FIN, KH), np.float32)
    for k in range(KH):
        Ssrc[k * DH:(k + 1) * DH, k] = a_src[k]
        Sdst[k * DH:(k + 1) * DH, k] = a_dst[k]

    sel = np.zeros((KH, KH * P), np.float32)
    for k in range(KH):
        sel[k, k * P:(k + 1) * P] = 1.0
    sel = sel.astype(bf)
    wu_host = np.ones((P, 8 + P), np.float32)
    wu_host[:, 8:] = np.eye(P, dtype=np.float32)
    wu_host = wu_host.astype(bf)

    maskB = (A > 0)  # [B, N, N] bool

    in_maps = []
    for c in range(8):
        b, half = divmod(c, 2)
        i0 = half * NI
        HbT = np.roll(H[b], -i0, axis=0).T
        maskT = np.ascontiguousarray(
            np.roll(maskB[b, i0:i0 + NI, :], -i0, axis=1).T
        ).astype(bf)
        cpack = np.concatenate(
            [W.astype(np.float32), Ssrc, Sdst, np.eye(P, dtype=np.float32), HbT],
            axis=1,
        ).astype(bf)
        in_maps.append({
            "maskT": maskT,
            "cpack": np.ascontiguousarray(cpack),
            "wu": wu_host,
            "sel": sel,
        })
    return in_maps


def kernel(H, A, W, a_src, a_dst, _want_results=False, _trace=False):
    H = np.asarray(H); A = np.asarray(A); W = np.asarray(W)
    a_src = np.asarray(a_src); a_dst = np.asarray(a_dst)

    if "nc" not in _CACHE:
        _CACHE["nc"] = _build_program()
    nc = _CACHE["nc"]

    in_maps = _host_prep(H, A, W, a_src, a_dst)
    res = run_bass_kernel_spmd(nc, in_maps, list(range(8)), trace=_trace)

    out = np.empty((B, N, KH * DH), np.float32)
    for c in range(8):
        b, half = divmod(c, 2)
        i0 = half * NI
        aux = res.results[c]["oaux"]  # [NIB, KH, DH+1, IBS]
        slab = aux[:, :, 0:DH, :] / aux[:, :, DH:DH + 1, :]
        # [ib, k, d, i] -> rows (ib*IBS + i), cols (k*DH + d)
        out[b, i0:i0 + NI, :] = (
            slab.transpose(0, 3, 1, 2).reshape(NI, KH * DH)
        )
    if _want_results:
        return out, res
    return out
